# revision 1
# baseline (speedup 1.0000x reference)
"""Trainium2 Bass kernel for nn_DNFLayer (fuzzy DNF layer).

Strategy
--------
Data-parallel over batch B=32 across 8 cores (4 batches/core). Per core the
(i, j) permutation grid is padded to the full 32x32 grid (diagonal masked via
the OR-kernel broadcast), giving 4096 rows = 32 row-tiles of 128 partitions.

The conjunct product over the 112 inputs is factorized per permutation
(i, j):  conj = F0(b) * FU1(b,i) * FU2(b,j) * FB1(b,i,j) * FB2(b,j,i),
each factor being a product of per-channel affine terms (alpha*x + beta)
with (alpha, beta) derived on-device from softmax(and_kernel). Products are
evaluated in the gamma form  prod(alpha x + beta) = prod(beta) * prod(gamma x
+ 1), gamma = alpha/beta, so the eval is one tensor_tensor multiply plus a
tensor_scalar (+1) and the per-(r,d) beta products fold into the OR-kernel
broadcast (computed exactly via Ln / matmul column-sum / Exp).

The heavy middle (eval + pairwise product trees) runs in bf16: every conjunct
is bounded above by exp(-21) ~ 7e-10 for any admissible input (and is ~e-45
in distribution), so a bf16 relative error there is invisible at fp32 output
resolution; the final probabilistic merges with the residual inputs are fp32.

Small broadcast constants are built on device with K=1 / K=32 matmuls (PE)
instead of large DMA broadcasts. Outputs are only the merged last channels;
the untouched channels of the output are the corresponding input slices.
"""

import numpy as np
import ml_dtypes

BF = ml_dtypes.bfloat16
B, N, P0, P1, P2, R, D = 32, 32, 16, 32, 16, 3, 8
RD = R * D              # 24
NCORE = 8
BL = B // NCORE         # 4 batches per core
NT = BL * 8             # 32 row-tiles of 128 per core

_CACHE = {}


def _build():
    import concourse.tile as tile
    from concourse import mybir, bacc

    F32 = mybir.dt.float32
    B16 = mybir.dt.bfloat16
    MUL = mybir.AluOpType.mult
    ADD = mybir.AluOpType.add
    SUB = mybir.AluOpType.subtract
    AF = mybir.ActivationFunctionType

    nc = bacc.Bacc("TRN2", target_bir_lowering=False, debug=False,
                   num_devices=NCORE)

    # ---- parameters (per-core shards / replicated constants) ----
    x_all_in = nc.declare_dram_parameter("x_all", [128, NT * 32], B16, isOutput=False)
    xu_in = nc.declare_dram_parameter("xu", [128, 80], B16, isOutput=False)
    akt_in = nc.declare_dram_parameter("akt", [112, 72], F32, isOutput=False)
    ork_in = nc.declare_dram_parameter("ork", [1, 24], F32, isOutput=False)
    sel_in = nc.declare_dram_parameter("selcat", [32, 1152], B16, isOutput=False)
    mask_in = nc.declare_dram_parameter("maskc", [128, 8], F32, isOutput=False)
    oldb_in = nc.declare_dram_parameter("olds_bin", [128, NT], F32, isOutput=False)
    oldu_in = nc.declare_dram_parameter("olds_un", [32, 4], F32, isOutput=False)
    oldn_in = nc.declare_dram_parameter("olds_null", [1, 4], F32, isOutput=False)

    out_binm = nc.declare_dram_parameter("out_binm", [128, NT], F32, isOutput=True)
    out_unm = nc.declare_dram_parameter("out_unm", [32, 4], F32, isOutput=True)
    out_nullm = nc.declare_dram_parameter("out_nullm", [1, 4], F32, isOutput=True)


    with tile.TileContext(nc) as tc:
        with tc.tile_pool(name="cb", bufs=1) as cb, \
             tc.tile_pool(name="wk", bufs=1) as wk, \
             tc.tile_pool(name="ps", bufs=3, space="PSUM") as ps, \
             tc.tile_pool(name="ps2", bufs=1, space="PSUM") as ps2:

            # ---------- input DMAs first (small/latency-critical first) ----------
            akt = cb.tile([112, 72], F32)
            nc.sync.dma_start(akt[:], akt_in[:])
            okt = cb.tile([1, 24], F32)
            nc.sync.dma_start(okt[:], okt_in := ork_in[:])
            xu = cb.tile([128, 80], B16)
            nc.sync.dma_start(xu[:], xu_in[:])
            maskc = cb.tile([128, 8], F32)
            nc.sync.dma_start(maskc[:], mask_in[:])
            sel = cb.tile([32, 1152], B16)
            nc.sync.dma_start(sel[:], sel_in[:])
            oldb = cb.tile([128, NT], F32)
            nc.gpsimd.dma_start(oldb[:], oldb_in[:])
            oldu = cb.tile([32, 4], F32)
            nc.gpsimd.dma_start(oldu[:], oldu_in[:])
            oldn = cb.tile([1, 4], F32)
            nc.gpsimd.dma_start(oldn[:], oldn_in[:])
            x_all = cb.tile([128, NT * 32], B16)
            for h in range(4):
                nc.sync.dma_start(x_all[:, h * 256:(h + 1) * 256],
                                  x_all_in[:, h * 256:(h + 1) * 256])

            # ---------- phase A: softmax -> gamma; ln(beta) sums ----------
            e = wk.tile([112, 72], F32)
            nc.scalar.activation(e[:], akt[:], AF.Exp)
            eok = wk.tile([1, 24], F32)
            nc.scalar.activation(eok[:], okt[:], AF.Exp, scale=-1.0)
            e3 = e[:].rearrange("p (r m) -> p r m", m=3)
            bsum = wk.tile([112, 24], F32)
            nc.vector.tensor_tensor(bsum[:], e3[:, :, 1], e3[:, :, 2], op=ADD)
            stot = wk.tile([112, 24], F32)
            nc.vector.tensor_tensor(stot[:], e3[:, :, 0], bsum[:], op=ADD)
            gam = wk.tile([112, 24], F32)
            nc.vector.tensor_tensor(gam[:], e3[:, :, 0], e3[:, :, 1], op=SUB)
            rbs = wk.tile([112, 24], F32)
            nc.vector.reciprocal(rbs[:], bsum[:])
            nc.vector.tensor_tensor(gam[:], gam[:], rbs[:], op=MUL)

            # transpose gamma on-chip: gamT[rd, k] (rows 24..31 / cols 112+ junk)
            gamP = wk.tile([128, 32], F32)
            nc.vector.memset(gamP[:], 1.0)
            nc.vector.tensor_copy(gamP[0:112, 0:24], gam[:])
            gamT = cb.tile([32, 128], F32)
            for blk in range(4):
                nc.vector.transpose(gamT[0:32, blk * 32:(blk + 1) * 32],
                                    gamP[blk * 32:(blk + 1) * 32, 0:32])

            lnb = wk.tile([112, 24], F32)
            nc.scalar.activation(lnb[:], bsum[:], AF.Ln)
            lns = wk.tile([112, 24], F32)
            nc.scalar.activation(lns[:], stot[:], AF.Ln)
            nc.vector.tensor_tensor(lnb[:], lnb[:], lns[:], op=SUB)
            ones112 = cb.tile([112, 1], F32)
            nc.vector.memset(ones112[:], 1.0)
            psb = ps.tile([1, 24], F32, tag="pp")
            nc.tensor.matmul(psb[:], ones112[:], lnb[:], start=True, stop=True)
            bA = wk.tile([1, 24], F32)
            nc.scalar.activation(bA[:], psb[:], AF.Exp)
            # sigmoid(ork) = 1 / (1 + exp(-ork)), exp shared with the table above
            sig = wk.tile([1, 24], F32)
            nc.vector.tensor_scalar(sig[:], eok[:], 1.0, None, op0=ADD)
            nc.vector.reciprocal(sig[:], sig[:])
            nc.vector.tensor_tensor(sig[:], sig[:], bA[:], op=MUL)

            # ---------- phase B: broadcast gamma consts via PE ----------
            ones1 = cb.tile([1, 128], F32)
            nc.vector.memset(ones1[:], 1.0)

            # binary: (rd, c32), k = 80+c
            g1 = cb.tile([1, 768], F32)
            nc.scalar.dma_start(g1[:].rearrange("p (r c) -> p r c", r=24),
                              gamT[0:24, 80:112])
            gB = cb.tile([128, 768], B16)
            for h in range(2):
                pst = ps.tile([128, 384], F32, tag="pp")
                nc.tensor.matmul(pst[:], ones1[:], g1[:, h * 384:(h + 1) * 384],
                                 start=True, stop=True)
                nc.vector.tensor_copy(gB[:, h * 384:(h + 1) * 384], pst[:])

            # unary: (seg2, rd, c32), k = 16 + seg*32 + c
            u1 = cb.tile([1, 1536], F32)
            nc.scalar.dma_start(u1[:].rearrange("p (r c) -> p r c", r=24),
                              gamT[0:24, 16:80])
            gUps = []
            for h in range(3):
                pst = ps2.tile([128, 512], F32, tag=f"gu{h}")
                nc.tensor.matmul(pst[:], ones1[:], u1[:, h * 512:(h + 1) * 512],
                                 start=True, stop=True)
                gUps.append(pst)

            # nullary: (rd, c16), k = c
            n1 = cb.tile([1, 384], F32)
            nc.scalar.dma_start(n1[:].rearrange("p (r c) -> p r c", r=24),
                              gamT[0:24, 0:16])
            gN = cb.tile([128, 384], B16)
            pst = ps.tile([128, 384], F32, tag="pp")
            nc.tensor.matmul(pst[:], ones1[:], n1[:], start=True, stop=True)
            nc.vector.tensor_copy(gN[:], pst[:])

            # ---------- phase C: unary/nullary factor pass ----------
            emU = wk.tile([128, 1536], B16)
            # flat (s, rd, c): chunk boundaries vs the s-split of x
            def _emu(dst_lo, n_grp, x_lo, gsrc, src_lo):
                nc.vector.tensor_tensor(
                    emU[:, dst_lo:dst_lo + n_grp * 32]
                        .rearrange("p (g c) -> p g c", c=32),
                    xu[:, x_lo:x_lo + 32].unsqueeze(1)
                        .broadcast_to((128, n_grp, 32)),
                    gsrc[:][:, src_lo:src_lo + n_grp * 32]
                        .rearrange("p (g c) -> p g c", c=32), op=MUL)
            _emu(0, 16, 0, gUps[0], 0)
            _emu(512, 8, 0, gUps[1], 0)
            _emu(768, 8, 32, gUps[1], 256)
            _emu(1024, 16, 32, gUps[2], 0)
            nc.scalar.activation(emU[:], emU[:], AF.Copy, bias=1.0)
            # U tree: [128, 48, 32] -> [128, 48]
            cur = emU[:].rearrange("p (g c) -> p g c", c=32)
            for w in (16, 8, 4, 2):
                nxt = wk.tile([128, 48 * w], B16, tag=f"ut{w}")
                nc.vector.tensor_tensor(
                    nxt[:].rearrange("p (g c) -> p g c", c=w),
                    cur[:, :, 0:w], cur[:, :, w:2 * w], op=MUL)
                cur = nxt[:].rearrange("p (g c) -> p g c", c=w)
            fu12 = wk.tile([128, 48], B16)
            nc.vector.tensor_tensor(fu12[:].unsqueeze(2), cur[:, :, 0:1],
                                    cur[:, :, 1:2], op=MUL)

            emN = wk.tile([128, 384], B16)
            nc.vector.tensor_tensor(
                emN[:].rearrange("p (r c) -> p r c", r=24),
                xu[:, 64:80].unsqueeze(1).broadcast_to((128, 24, 16)),
                gN[:].rearrange("p (r c) -> p r c", r=24), op=MUL)
            nc.scalar.activation(emN[:], emN[:], AF.Copy, bias=1.0)
            cur = emN[:].rearrange("p (g c) -> p g c", c=16)
            for w in (8, 4, 2):
                nxt = wk.tile([128, 24 * w], B16, tag=f"nt{w}")
                nc.vector.tensor_tensor(
                    nxt[:].rearrange("p (g c) -> p g c", c=w),
                    cur[:, :, 0:w], cur[:, :, w:2 * w], op=MUL)
                cur = nxt[:].rearrange("p (g c) -> p g c", c=w)
            f0g = wk.tile([128, 24], B16)
            nc.vector.tensor_tensor(f0g[:].unsqueeze(2), cur[:, :, 0:1],
                                    cur[:, :, 1:2], op=MUL)

            fu2f0 = wk.tile([128, 24], B16)
            nc.vector.tensor_tensor(fu2f0[:], fu12[:, 24:48], f0g[:], op=MUL)

            # ---------- phase D: per-b row broadcasts via PE ----------
            FU1B = cb.tile([128, 768], B16)
            FU2F0B = cb.tile([128, 96], B16)
            for b in range(BL):
                rhs1 = wk.tile([32, 24], B16, tag="rhs1")
                nc.vector.tensor_copy(rhs1[:], fu12[b * 32:(b + 1) * 32, 0:24])
                rhs2 = wk.tile([32, 24], B16, tag="rhs2")
                nc.vector.tensor_copy(rhs2[:], fu2f0[b * 32:(b + 1) * 32, :])
                psF = ps.tile([128, 192], F32, tag="pp")
                for t in range(8):
                    nc.tensor.matmul(psF[:, t * 24:(t + 1) * 24],
                                     sel[0:32, t * 128:(t + 1) * 128],
                                     rhs1[:], start=True, stop=True)
                nc.vector.tensor_copy(FU1B[:, b * 192:(b + 1) * 192], psF[:])
                psJ = ps.tile([128, 24], F32, tag="pp")
                nc.tensor.matmul(psJ[:], sel[0:32, 1024:1152],
                                 rhs2[:], start=True, stop=True)
                nc.vector.tensor_copy(FU2F0B[:, b * 24:(b + 1) * 24], psJ[:])

            # OR kernel broadcast * diag mask (sig computed in phase A)
            psO = ps.tile([128, 24], F32, tag="pp")
            nc.tensor.matmul(psO[:], ones1[:], sig[:], start=True, stop=True)
            okmB = cb.tile([128, 192], B16)
            for t in range(8):
                nc.vector.tensor_scalar(okmB[:, t * 24:(t + 1) * 24], psO[:],
                                        maskc[:, t:t + 1], None, op0=MUL)

            # PFOK[p, (b,t,rd)] = FU1B * FU2F0B(bcast t) * okmB(bcast b)
            PFOK = cb.tile([128, 768], B16)
            nc.vector.tensor_tensor(
                PFOK[:].rearrange("p (b t r) -> p b t r", b=4, t=8),
                FU1B[:].rearrange("p (b t r) -> p b t r", b=4, t=8),
                FU2F0B[:].rearrange("p (b r) -> p b r", b=4)
                    .unsqueeze(2).broadcast_to((128, 4, 8, 24)), op=MUL)
            nc.vector.tensor_tensor(
                PFOK[:].rearrange("p (b t r) -> p b t r", b=4, t=8),
                PFOK[:].rearrange("p (b t r) -> p b t r", b=4, t=8),
                okmB[:].rearrange("p (t r) -> p t r", t=8)
                    .unsqueeze(1).broadcast_to((128, 4, 8, 24)), op=MUL)

            # ---------- phase E: main binary pipeline (emitted early) ----------
            em = wk.tile([128, NT * 768], B16)
            t1 = wk.tile([128, NT * 384], B16)
            t2 = wk.tile([128, NT * 192], B16)
            t3 = wk.tile([128, NT * 96], B16)
            t4 = wk.tile([128, NT * 48], B16)
            cj = wk.tile([128, NT * 24], B16)
            gA = wk.tile([128, 768], B16)
            d1 = wk.tile([128, 384], B16)
            d2 = wk.tile([128, 192], B16)
            pdA2 = wk.tile([128, 96], B16)
            for b in range(BL):
                nc.vector.tensor_tensor(
                    em[:, b * 6144:(b + 1) * 6144]
                        .rearrange("p (k r c) -> p k r c", k=8, r=24),
                    x_all[:, b * 256:(b + 1) * 256]
                        .rearrange("p (k c) -> p k c", k=8)
                        .unsqueeze(2).broadcast_to((128, 8, 24, 32)),
                    gB[:].rearrange("p (r c) -> p r c", r=24)
                        .unsqueeze(1).broadcast_to((128, 8, 24, 32)), op=MUL)
                if b == BL - 1:
                    nc.vector.tensor_scalar(em[:, b * 6144:(b + 1) * 6144],
                                            em[:, b * 6144:(b + 1) * 6144],
                                            1.0, None, op0=ADD)
                else:
                    nc.scalar.activation(em[:, b * 6144:(b + 1) * 6144],
                                         em[:, b * 6144:(b + 1) * 6144],
                                         AF.Copy, bias=1.0)
                cur = em[:, b * 6144:(b + 1) * 6144].rearrange(
                    "p (g c) -> p g c", c=32)
                for w, tl in ((16, t1), (8, t2), (4, t3), (2, t4)):
                    dst = tl[:, b * 192 * w:(b + 1) * 192 * w].rearrange(
                        "p (g c) -> p g c", c=w)
                    nc.vector.tensor_tensor(dst, cur[:, :, 0:w],
                                            cur[:, :, w:2 * w], op=MUL)
                    cur = dst
                nc.vector.tensor_tensor(
                    cj[:, b * 192:(b + 1) * 192].unsqueeze(2),
                    cur[:, :, 0:1], cur[:, :, 1:2], op=MUL)
                cjb = cj[:, b * 192:(b + 1) * 192]
                nc.vector.tensor_tensor(cjb, cjb, PFOK[:, b * 192:(b + 1) * 192],
                                        op=MUL)
                gAb = gA[:, b * 192:(b + 1) * 192]
                nc.vector.tensor_scalar(gAb, cjb, -1.0, 1.0, op0=MUL, op1=ADD)
                d1b = d1[:, b * 96:(b + 1) * 96].rearrange(
                    "p (g dd) -> p g dd", dd=4)
                gvb = gAb.rearrange("p (g dd) -> p g dd", dd=8)
                nc.vector.tensor_tensor(d1b, gvb[:, :, 0:4], gvb[:, :, 4:8],
                                        op=MUL)
                d2b = d2[:, b * 48:(b + 1) * 48].rearrange(
                    "p (g dd) -> p g dd", dd=2)
                nc.vector.tensor_tensor(d2b, d1b[:, :, 0:2], d1b[:, :, 2:4],
                                        op=MUL)
                # write (r, k)-ordered pd: pdA2[p, r*32 + b*8 + t]
                d2b4 = d2[:, b * 48:(b + 1) * 48].rearrange(
                    "p (t r dd) -> p t r dd", t=8, r=3)
                nc.vector.tensor_tensor(
                    pdA2[:].rearrange("p (r k) -> p r k", r=3)
                        [:, :, b * 8:(b + 1) * 8]
                        .transpose([0, 2, 1]).unsqueeze(3),
                    d2b4[:, :, :, 0:1], d2b4[:, :, :, 1:2], op=MUL)

            pdF = wk.tile([128, 96], F32)
            nc.vector.tensor_copy(pdF[:], pdA2[:])

            # ---------- phase F: merges (all on-chip) ----------
            # binary last channel (row layout: [128, NT]); pd r=2 block
            tb = wk.tile([128, NT], F32)
            nc.vector.tensor_scalar(tb[:], oldb[:], -1.0, 1.0, op0=MUL, op1=ADD)
            nc.vector.tensor_tensor(tb[:], tb[:], pdF[:, 64:96], op=MUL)
            nc.vector.tensor_scalar(tb[:], tb[:], -1.0, 1.0, op0=MUL, op1=ADD)
            nc.sync.dma_start(out_binm[:], tb[:])

            # transpose r=0 / r=1 pd blocks to [32 rows=(b,i8), 128=(i4,j)]
            r1T = wk.tile([32, 128], F32)
            r0T = wk.tile([32, 128], F32)
            for blk in range(4):
                nc.vector.transpose(r1T[0:32, blk * 32:(blk + 1) * 32],
                                    pdF[blk * 32:(blk + 1) * 32, 32:64])
                nc.vector.transpose(r0T[0:32, blk * 32:(blk + 1) * 32],
                                    pdF[blk * 32:(blk + 1) * 32, 0:32])

            # unary: product over j within each (b, i8, i4)
            cur = r1T[:].rearrange("p (i4 j) -> p i4 j", i4=4)
            for w in (16, 8, 4, 2, 1):
                nxt = wk.tile([32, 4 * w], F32, tag=f"pu{w}")
                nxtv = nxt[:].rearrange("p (i4 j) -> p i4 j", i4=4)
                nc.vector.tensor_tensor(nxtv, cur[:, :, 0:w], cur[:, :, w:2 * w],
                                        op=MUL)
                cur = nxtv
            pdu = cur.rearrange("p i4 j -> p (i4 j)")  # [32, 4]
            tu = wk.tile([32, 4], F32)
            nc.vector.tensor_scalar(tu[:], oldu[:], -1.0, 1.0, op0=MUL, op1=ADD)
            nc.vector.tensor_tensor(tu[:], tu[:], pdu, op=MUL)
            nc.vector.tensor_scalar(tu[:], tu[:], -1.0, 1.0, op0=MUL, op1=ADD)
            nc.sync.dma_start(out_unm[:], tu[:])

            # nullary: product over all (i, j) per b
            cur = r0T[:]
            for w in (64, 32, 16, 8, 4, 2, 1):
                nxt = wk.tile([32, w], F32, tag=f"pn{w}")
                nc.vector.tensor_tensor(nxt[:], cur[:, 0:w], cur[:, w:2 * w],
                                        op=MUL)
                cur = nxt[:]
            # fold the remaining 32 partition values (b, i8) -> per-b products
            q = wk.tile([32, 32], F32)
            nc.vector.memset(q[:], 1.0)
            nc.vector.tensor_copy(q[:, 0:1], cur)
            qT = wk.tile([32, 32], F32)
            nc.vector.transpose(qT[:], q[:])
            cur = qT[0:1, :].rearrange("p (b i8) -> p b i8", b=4)
            for w in (4, 2, 1):
                nxt = wk.tile([1, 4 * w], F32, tag=f"pq{w}")
                nxtv = nxt[:].rearrange("p (b i8) -> p b i8", b=4)
                nc.vector.tensor_tensor(nxtv, cur[:, :, 0:w], cur[:, :, w:2 * w],
                                        op=MUL)
                cur = nxtv
            pdn = cur.rearrange("p b i8 -> p (b i8)")  # [1, 4]
            tn = wk.tile([1, 4], F32)
            nc.vector.tensor_scalar(tn[:], oldn[:], -1.0, 1.0, op0=MUL, op1=ADD)
            nc.vector.tensor_tensor(tn[:], tn[:], pdn, op=MUL)
            nc.vector.tensor_scalar(tn[:], tn[:], -1.0, 1.0, op0=MUL, op1=ADD)
            nc.sync.dma_start(out_nullm[:], tn[:])

    nc.compile()
    return nc


def _host_prep(nullary_preds, unary_preds, binary_preds, and_kernel, or_kernel):
    """Build per-core input maps (sharding + layout prep only)."""
    null_ = np.asarray(nullary_preds, np.float32)
    un = np.asarray(unary_preds, np.float32)
    bi = np.asarray(binary_preds, np.float32)
    ak = np.asarray(and_kernel, np.float32)
    ok = np.asarray(or_kernel, np.float32)

    I, J = np.meshgrid(np.arange(N), np.arange(N), indexing="ij")
    off = I != J
    Jm = J - (J > I)
    Im = I - (I > J)

    binP = np.zeros((B, N, N, P2), np.float32)
    binP[:, off] = bi[:, I[off], Jm[off]]
    binT = np.zeros((B, N, N, P2), np.float32)
    binT[:, off] = bi[:, J[off], Im[off]]
    binPT = np.concatenate([binP, binT], axis=-1)          # [B,32,32,32]

    # row-tile layout: x_all[core][p, k=(b,t), c] = binPT[4c+b, t*128+p, c]
    xg = binPT.reshape(NCORE, BL, 8, 128, 32)
    x_all = np.ascontiguousarray(xg.transpose(0, 3, 1, 2, 4)
                                 ).reshape(NCORE, 128, NT * 32).astype(BF)
    olds_bin = np.ascontiguousarray(
        binP[..., 15].reshape(NCORE, BL, 8, 128).transpose(0, 3, 1, 2)
    ).reshape(NCORE, 128, NT).astype(np.float32)

    # unary pass rows (b, i): [u | u | n]
    xun = np.concatenate(
        [un, un, np.broadcast_to(null_[:, None, :], (B, N, P0))], axis=-1)
    xu = xun.reshape(NCORE, 128, 80).astype(BF)
    # rows (b, i8), cols i4 : out_unm[q=(b*8+i//4), i%4]
    olds_un = un[..., 31].reshape(NCORE, 4, 8, 4).reshape(NCORE, 32, 4).astype(np.float32)
    olds_null = null_[:, 15].reshape(NCORE, 1, 4).astype(np.float32)

    akT = np.ascontiguousarray(ak.transpose(2, 0, 1, 3)).reshape(112, 72)
    ork = ok.reshape(1, 24).astype(np.float32)

    p = np.arange(128)
    t = np.arange(8)
    selT = (np.arange(32)[:, None, None] == (t[None, :, None] * 4 + p[None, None, :] // 32))
    selJ = (np.arange(32)[:, None] == (p[None, :] % 32))
    selcat = np.concatenate([selT.reshape(32, 1024), selJ], axis=1).astype(BF)
    maskc = ((p[:, None] % 32) != (t[None, :] * 4 + p[:, None] // 32)
             ).astype(np.float32)

    in_maps = []
    for c in range(NCORE):
        in_maps.append({
            "x_all": x_all[c],
            "xu": xu[c],
            "akt": akT,
            "ork": ork,
            "selcat": selcat,
            "maskc": maskc,
            "olds_bin": olds_bin[c],
            "olds_un": olds_un[c],
            "olds_null": olds_null[c],
        })
    return in_maps


def _assemble(results, nullary_preds, unary_preds, binary_preds):
    null_ = np.asarray(nullary_preds, np.float32).copy()
    un = np.asarray(unary_preds, np.float32).copy()
    bi = np.asarray(binary_preds, np.float32).copy()

    I, J = np.meshgrid(np.arange(N), np.arange(N), indexing="ij")
    off = I != J
    Jm = J - (J > I)

    for c in range(NCORE):
        r = results[c]
        # out_binm [128, NT=(b,t)] -> rows[b, t*128+p]
        ob = r["out_binm"].reshape(128, BL, 8).transpose(1, 2, 0).reshape(BL, N, N)
        for bl in range(BL):
            b = c * BL + bl
            bi[b, I[off], Jm[off], 15] = ob[bl][off]
        un[c * BL:(c + 1) * BL, :, 31] = r["out_unm"].reshape(BL, 8, 4).reshape(BL, N)
        null_[c * BL:(c + 1) * BL, 15] = r["out_nullm"].reshape(BL)

    return np.concatenate(
        [null_, un.reshape(B, -1), bi.reshape(B, -1)], axis=-1)


def kernel(nullary_preds, unary_preds, binary_preds, and_kernel, or_kernel):
    from concourse.bass_utils import run_bass_kernel_spmd

    if "nc" not in _CACHE:
        _CACHE["nc"] = _build()
    nc = _CACHE["nc"]

    in_maps = _host_prep(nullary_preds, unary_preds, binary_preds,
                         and_kernel, or_kernel)
    res = run_bass_kernel_spmd(nc, in_maps, list(range(NCORE)))
    return _assemble(res.results, nullary_preds, unary_preds, binary_preds)


if __name__ == "__main__":
    import reference as ref
    ins = {k: np.asarray(v) for k, v in ref.setup_inputs().items()}
    out = kernel(**ins)
    print("kernel out:", out.shape, out.dtype)

